# revision 2
# baseline (speedup 1.0000x reference)
"""Trainium2 Bass kernel for the NeuralODE adjoint problem.

Exploits that the reference scan's carry passes the *observed* Y[s] to the
next segment, so all B*S = 128 (sample, segment) tasks are independent.
Each of 8 cores handles 16 units (2 samples x 8 segments), batched along
the matmul free dimension in feature-major layout.

Per unit: RK4 forward (4 steps x 4 stages) for y1_hat, then RK4 backward
over the augmented adjoint ODE. Gradient outer products are deferred: the
backward loop stacks stage values (y, H=tanh(u), Ehat, a) into wide SBUF
buffers; at the end 8 PE transposes + 6 matmuls produce all parameter
grads, accumulated over (stage, unit) as the contraction dim.

Scaling tricks (validated vs reference in numpy to ~2e-7 rel):
  Ehat = (w_i*dt/6) * (1-H^2) * (W2 @ a)   [per-unit scale folded via
         host-precomputed broadcast tiles; sign via (HH-1)*C * (-s)]
  DA   = W1 @ Ehat = -s*da  =>  a_stage = a0 + (6c/w)*DA (unit-uniform!)
         a_new = a0 + sum_i DA_i  (PSUM-accumulated)
"""
import numpy as np

N, Hd, B, S, NS = 64, 128, 16, 8, 4
NU = 16          # units per core
NC = 8           # cores
WID = 256        # stack width = 16 slices * 16 units
RKW = np.array([1.0, 2.0, 2.0, 1.0], dtype=np.float32)
ACONST = [3.0, 1.5, 3.0]  # 6*c_{i+1}/w_i for i=0,1,2

# column offsets inside the two constant blobs
C128 = {"w2": 0, "w1r": 64, "b1": 128, "i128": 129, "nsb": 257, "tbc": 289}
W128 = 291
C64 = {"w2r": 0, "y1t2": 128, "dtbc": 144, "saw": 240}
W64 = 496

_CACHE = {}


def _f32(x):
    return np.asarray(x, dtype=np.float32)


def _build_program():
    import contextlib

    import concourse.bacc as bacc
    import concourse.bass as bass
    import concourse.mybir as mybir
    import concourse.tile as tile

    F32 = mybir.dt.float32
    Alu = mybir.AluOpType
    Act = mybir.ActivationFunctionType
    ts = bass.ts

    nc = bacc.Bacc("TRN2", target_bir_lowering=False, debug=False, num_devices=NC)

    d_b128 = nc.dram_tensor("b128", [128, W128], F32, kind="ExternalInput").ap()
    d_b64 = nc.dram_tensor("b64", [64, W64], F32, kind="ExternalInput").ap()
    d_w1vt = nc.dram_tensor("w1vt", [65, 128], F32, kind="ExternalInput").ap()
    d_y0t = nc.dram_tensor("y0t", [64, NU], F32, kind="ExternalInput").ap()
    d_tf = nc.dram_tensor("tf", [1, WID], F32, kind="ExternalInput").ap()
    d_tb = nc.dram_tensor("tb", [1, WID], F32, kind="ExternalInput").ap()

    d_yhat = nc.dram_tensor("yhat", [64, NU], F32, kind="ExternalOutput").ap()
    d_gw1 = nc.dram_tensor("gw1", [64, 128], F32, kind="ExternalOutput").ap()
    d_gw2 = nc.dram_tensor("gw2", [128, 64], F32, kind="ExternalOutput").ap()
    d_gvt = nc.dram_tensor("gvt", [128, 1], F32, kind="ExternalOutput").ap()
    d_gb1 = nc.dram_tensor("gb1", [128, 1], F32, kind="ExternalOutput").ap()
    d_gb2 = nc.dram_tensor("gb2", [64, 1], F32, kind="ExternalOutput").ap()

    with tile.TileContext(nc) as tc, contextlib.ExitStack() as ctx:
        cp = ctx.enter_context(tc.tile_pool(name="cp", bufs=1))

        B128 = cp.tile([128, W128], F32)
        nc.sync.dma_start(B128[:], d_b128)
        B64 = cp.tile([64, W64], F32)
        nc.sync.dma_start(B64[:], d_b64)
        W1VT = cp.tile([65, 128], F32)
        nc.sync.dma_start(W1VT[:], d_w1vt)

        def c128(key, w, p=128):
            o = C128[key]
            return B128[0:p, o:o + w]

        def c64(key, off, w):
            o = C64[key] + off
            return B64[:, o:o + w]

        W2 = c128("w2", 64)
        W1R = c128("w1r", 64)
        B1 = c128("b1", 1)
        I128 = c128("i128", 128)
        I64 = c128("i128", 64, p=64)
        NS1 = c128("nsb", 16)
        NS2 = B128[:, C128["nsb"] + 16:C128["nsb"] + 32]
        TBC = c128("tbc", 2)
        W2R = c64("w2r", 0, 128)
        Y1T2 = c64("y1t2", 0, 16)
        DT2 = c64("dtbc", 0, 16)
        DT = c64("dtbc", 16, 16)
        DT6 = c64("dtbc", 32, 16)
        BDT2 = c64("dtbc", 48, 16)
        BDT = c64("dtbc", 64, 16)
        BDT6 = c64("dtbc", 80, 16)
        SAW = c64("saw", 0, 256)
        B2 = B64[:, C64["y1t2"] + 16:C64["y1t2"] + 17]  # stored next to y1t2

        YSF = cp.tile([65, WID], F32)
        nc.sync.dma_start(YSF[64:65, :], d_tf)
        nc.sync.dma_start(YSF[0:64, 0:NU], d_y0t)
        YSB = cp.tile([65, WID], F32)
        nc.sync.dma_start(YSB[64:65, :], d_tb)
        HST = cp.tile([128, WID], F32)
        EST = cp.tile([128, WID], F32)
        AST = cp.tile([64, WID], F32)

        with tc.tile_pool(name="ps", bufs=1, space="PSUM") as ps, \
             tc.tile_pool(name="wk", bufs=3) as wk, \
             tc.tile_pool(name="kp", bufs=6) as kp, \
             tc.tile_pool(name="hp", bufs=3) as hp:

            def ynew(ks, base_y, dt6ap, dest):
                s1 = wk.tile([64, NU], F32, tag="s1")
                nc.vector.tensor_tensor(s1[:], ks[0][:], ks[3][:], op=Alu.add)
                s2 = wk.tile([64, NU], F32, tag="s2")
                nc.vector.tensor_tensor(s2[:], ks[1][:], ks[2][:], op=Alu.add)
                ss = wk.tile([64, NU], F32, tag="ss")
                nc.vector.scalar_tensor_tensor(
                    ss[:], s2[:], 2.0, s1[:], Alu.mult, Alu.add)
                t2 = wk.tile([64, NU], F32, tag="t2")
                nc.vector.tensor_tensor(t2[:], ss[:], dt6ap, op=Alu.mult)
                nc.vector.tensor_tensor(dest, t2[:], base_y, op=Alu.add)

            # ---------------- forward ----------------
            for m in range(NS):
                base_y = YSF[0:64, ts(4 * m, NU)]
                ks = []
                for i in range(4):
                    sl = 4 * m + i
                    s16 = ts(sl, NU)
                    U = ps.tile([128, NU], F32, tag="u")
                    nc.tensor.matmul(U[:], W1VT[:], YSF[:, s16],
                                     start=True, stop=True)
                    Hf = hp.tile([128, NU], F32, tag="h")
                    nc.scalar.activation(Hf[:], U[:], Act.Tanh, bias=B1)
                    Fp = ps.tile([64, NU], F32, tag="f")
                    nc.tensor.matmul(Fp[:], W2, Hf[:], start=True, stop=True)
                    k = kp.tile([64, NU], F32, tag="k")
                    nc.scalar.activation(k[:], Fp[:], Act.Identity, bias=B2)
                    ks.append(k)
                    if i < 3:
                        tmp = wk.tile([64, NU], F32, tag="tmp")
                        nc.vector.tensor_tensor(
                            tmp[:], k[:], DT2 if i < 2 else DT, op=Alu.mult)
                        nc.vector.tensor_tensor(
                            YSF[0:64, ts(sl + 1, NU)], tmp[:], base_y, op=Alu.add)
                dest = (YSF[0:64, ts(4 * (m + 1), NU)] if m < 3
                        else YSB[0:64, 0:NU])
                ynew(ks, base_y, DT6, dest)

            nc.sync.dma_start(d_yhat, YSB[0:64, 0:NU])
            # a0 = 2*yhat - 2*y1_obs
            nc.vector.scalar_tensor_tensor(
                AST[:, 0:NU], YSB[0:64, 0:NU], 2.0, Y1T2, Alu.mult, Alu.subtract)

            # ---------------- backward ----------------
            for m in range(NS):
                base_y = YSB[0:64, ts(4 * m, NU)]
                base_a = AST[:, ts(4 * m, NU)]
                if m < 3:
                    DACC = ps.tile([64, NU], F32, tag="dacc")
                ks = []
                for i in range(4):
                    sl = 4 * m + i
                    s16 = ts(sl, NU)
                    U = ps.tile([128, NU], F32, tag="u")
                    nc.tensor.matmul(U[:], W1VT[:], YSB[:, s16],
                                     start=True, stop=True)
                    nc.scalar.activation(HST[:, s16], U[:], Act.Tanh, bias=B1)
                    Cc = ps.tile([128, NU], F32, tag="c")
                    nc.tensor.matmul(Cc[:], W2R, AST[:, s16],
                                     start=True, stop=True)
                    HH = wk.tile([128, NU], F32, tag="hh")
                    nc.scalar.activation(HH[:], HST[:, s16], Act.Square)
                    nE = wk.tile([128, NU], F32, tag="ne")
                    nc.vector.scalar_tensor_tensor(
                        nE[:], HH[:], 1.0, Cc[:], Alu.subtract, Alu.mult)
                    nc.vector.tensor_tensor(
                        EST[:, s16], nE[:], NS1 if i in (0, 3) else NS2,
                        op=Alu.mult)
                    if i < 3 or m < 3:
                        Fp = ps.tile([64, NU], F32, tag="f")
                        nc.tensor.matmul(Fp[:], W2, HST[:, s16],
                                         start=True, stop=True)
                        k = kp.tile([64, NU], F32, tag="k")
                        nc.scalar.activation(k[:], Fp[:], Act.Identity, bias=B2)
                        ks.append(k)
                    if i < 3:
                        DA = ps.tile([64, NU], F32, tag="da")
                        nc.tensor.matmul(DA[:], W1R, EST[:, s16],
                                         start=True, stop=True)
                    if m < 3:
                        nc.tensor.matmul(DACC[:], W1R, EST[:, s16],
                                         start=(i == 0), stop=(i == 3),
                                         skip_group_check=True)
                    if i < 3:
                        tmp = wk.tile([64, NU], F32, tag="tmp")
                        nc.vector.tensor_tensor(
                            tmp[:], k[:], BDT2 if i < 2 else BDT, op=Alu.mult)
                        nc.vector.tensor_tensor(
                            YSB[0:64, ts(sl + 1, NU)], tmp[:], base_y,
                            op=Alu.add)
                        nc.vector.scalar_tensor_tensor(
                            AST[:, ts(sl + 1, NU)], DA[:], ACONST[i], base_a,
                            Alu.mult, Alu.add)
                if m < 3:
                    ynew(ks, base_y, BDT6, YSB[0:64, ts(4 * (m + 1), NU)])
                    nc.vector.tensor_tensor(
                        AST[:, ts(4 * (m + 1), NU)], base_a, DACC[:],
                        op=Alu.add)

        # ---------------- gradients ----------------
        with tc.tile_pool(name="pg", bufs=1, space="PSUM") as pg, \
             tc.tile_pool(name="um", bufs=1) as um:
            ASW = um.tile([64, WID], F32)
            nc.vector.tensor_tensor(ASW[:], AST[:], SAW, op=Alu.mult)
            GB1 = um.tile([128, 1], F32)
            nc.vector.tensor_reduce(GB1[:], EST[:], axis=mybir.AxisListType.X,
                                    op=Alu.add)
            nc.sync.dma_start(d_gb1, GB1[:])
            GB2 = um.tile([64, 1], F32)
            nc.vector.tensor_reduce(GB2[:], ASW[:], axis=mybir.AxisListType.X,
                                    op=Alu.add)
            nc.sync.dma_start(d_gb2, GB2[:])

            yum, eum, hum, aum = [], [], [], []
            for c in range(2):
                cols = ts(c, 128)
                TY = pg.tile([128, 64], F32, tag="tp", bufs=2)
                nc.tensor.transpose(TY[:], YSB[0:64, cols], I64)
                Y_ = um.tile([128, 64], F32, tag="yum", bufs=2)
                nc.scalar.activation(Y_[:], TY[:], Act.Copy)
                yum.append(Y_)
                TE = pg.tile([128, 128], F32, tag="tp2", bufs=2)
                nc.tensor.transpose(TE[:], EST[:, cols], I128)
                E_ = um.tile([128, 128], F32, tag="eum", bufs=2)
                nc.scalar.activation(E_[:], TE[:], Act.Copy)
                eum.append(E_)
                TH = pg.tile([128, 128], F32, tag="tp2", bufs=2)
                nc.tensor.transpose(TH[:], HST[:, cols], I128)
                H_ = um.tile([128, 128], F32, tag="hum", bufs=2)
                nc.scalar.activation(H_[:], TH[:], Act.Copy)
                hum.append(H_)
                TA = pg.tile([128, 64], F32, tag="tp", bufs=2)
                nc.tensor.transpose(TA[:], ASW[:, cols], I64)
                A_ = um.tile([128, 64], F32, tag="aum", bufs=2)
                nc.scalar.activation(A_[:], TA[:], Act.Copy)
                aum.append(A_)

            GW1P = pg.tile([64, 128], F32, tag="gw1")
            GW2P = pg.tile([128, 64], F32, tag="gw2")
            GVTP = pg.tile([128, 1], F32, tag="gvt")
            for c in range(2):
                nc.tensor.matmul(GW1P[:], yum[c][:], eum[c][:],
                                 start=(c == 0), stop=(c == 1))
                nc.tensor.matmul(GW2P[:], hum[c][:], aum[c][:],
                                 start=(c == 0), stop=(c == 1))
                nc.tensor.matmul(GVTP[:], eum[c][:], TBC[:, c:c + 1],
                                 start=(c == 0), stop=(c == 1))
            GW1S = um.tile([64, 128], F32)
            nc.scalar.activation(GW1S[:], GW1P[:], Act.Copy)
            nc.sync.dma_start(d_gw1, GW1S[:])
            GW2S = um.tile([128, 64], F32)
            nc.scalar.activation(GW2S[:], GW2P[:], Act.Copy)
            nc.sync.dma_start(d_gw2, GW2S[:])
            GVTS = um.tile([128, 1], F32)
            nc.scalar.activation(GVTS[:], GVTP[:], Act.Copy)
            nc.sync.dma_start(d_gvt, GVTS[:])

    nc.compile()
    return nc


def _host_prep(Ys, Ts, W1, b1, vt, W2, b2):
    """Build per-core input maps (all exact-fp32 mirroring the reference)."""
    Ys, Ts = _f32(Ys), _f32(Ts)
    W1, b1, vt, W2, b2 = map(_f32, (W1, b1, vt, W2, b2))

    w1vt = np.vstack([W1, vt[None, :]])                        # (65,128)

    in_maps = []
    for core in range(NC):
        bs = [2 * core, 2 * core + 1]
        y0 = np.zeros((NU, N), np.float32)
        y1 = np.zeros((NU, N), np.float32)
        t0 = np.zeros(NU, np.float32)
        t1 = np.zeros(NU, np.float32)
        for ib, b in enumerate(bs):
            for s in range(S):
                u = ib * S + s
                sp = max(s - 1, 0)
                y0[u], t0[u] = Ys[b, sp], Ts[b, sp]
                y1[u], t1[u] = Ys[b, s], Ts[b, s]
        dt = _f32((t1 - t0) / NS)
        dtb = _f32(-dt)

        def stage_times(tstart, step):
            out = np.zeros((NS, 4, NU), np.float32)
            tcur = tstart.copy()
            half = _f32(step * _f32(0.5))
            for m in range(NS):
                out[m, 0] = tcur
                out[m, 1] = _f32(tcur + half)
                out[m, 2] = out[m, 1]
                out[m, 3] = _f32(tcur + step)
                tcur = _f32(tcur + step)
            return out.reshape(16, NU)

        tf = stage_times(t0, dt).reshape(1, WID)
        tbm = stage_times(t1, dtb)
        tb = tbm.reshape(1, WID)
        tbc = tb.reshape(2, 128).T.copy()                      # (128,2)

        dt6 = _f32(dt / 6)
        blocks = [_f32(dt * _f32(0.5)), dt, dt6,
                  _f32(dtb * _f32(0.5)), dtb, _f32(dtb / 6)]
        dtbc = np.concatenate(
            [np.broadcast_to(x, (N, NU)) for x in blocks], axis=1)  # (64,96)
        saw = np.concatenate(
            [np.broadcast_to(_f32(RKW[i] * dt6), (N, NU))
             for m in range(NS) for i in range(4)], axis=1)        # (64,256)
        ns1 = np.broadcast_to(_f32(-dt6), (Hd, NU))
        ns2 = np.broadcast_to(_f32(-dt6 * 2), (Hd, NU))

        b128 = np.zeros((128, W128), np.float32)
        b128[:, C128["w2"]:C128["w2"] + 64] = W2
        b128[:, C128["w1r"]:C128["w1r"] + 64] = W1.T
        b128[:, C128["b1"]] = b1
        b128[:, C128["i128"]:C128["i128"] + 128] = np.eye(128, dtype=np.float32)
        b128[:, C128["nsb"]:C128["nsb"] + 16] = ns1
        b128[:, C128["nsb"] + 16:C128["nsb"] + 32] = ns2
        b128[:, C128["tbc"]:C128["tbc"] + 2] = tbc

        b64 = np.zeros((64, W64), np.float32)
        b64[:, C64["w2r"]:C64["w2r"] + 128] = W2.T
        b64[:, C64["y1t2"]:C64["y1t2"] + 16] = _f32(2.0 * y1.T)
        b64[:, C64["y1t2"] + 16] = b2
        b64[:, C64["dtbc"]:C64["dtbc"] + 96] = dtbc
        b64[:, C64["saw"]:C64["saw"] + 256] = saw

        in_maps.append({
            "b128": b128, "b64": b64, "w1vt": w1vt,
            "y0t": np.ascontiguousarray(y0.T), "tf": tf, "tb": tb,
        })
    return in_maps


def _run(inputs, trace=False, tmpdir=None):
    from concourse.bass_utils import run_bass_kernel_spmd

    if "nc" not in _CACHE:
        _CACHE["nc"] = _build_program()
    nc = _CACHE["nc"]
    in_maps = _host_prep(**inputs)
    res = run_bass_kernel_spmd(nc, in_maps, list(range(NC)), trace=trace,
                               tmpdir=tmpdir)

    Ys_hat = np.zeros((B, S, N), np.float32)
    gW1 = np.zeros((N, Hd), np.float32)
    gW2 = np.zeros((Hd, N), np.float32)
    gvt = np.zeros(Hd, np.float32)
    gb1 = np.zeros(Hd, np.float32)
    gb2 = np.zeros(N, np.float32)
    for c in range(NC):
        r = res.results[c]
        Ys_hat[2 * c:2 * c + 2] = r["yhat"].T.reshape(2, S, N)
        gW1 += r["gw1"]
        gW2 += r["gw2"]
        gvt += r["gvt"][:, 0]
        gb1 += r["gb1"][:, 0]
        gb2 += r["gb2"][:, 0]
    return (Ys_hat, gW1, gb1, gvt, gW2, gb2), res


def kernel(Ys, Ts, W1, b1, vt, W2, b2):
    out, _ = _run(dict(Ys=Ys, Ts=Ts, W1=W1, b1=b1, vt=vt, W2=W2, b2=b2))
    return out


# revision 10
# speedup vs baseline: 1.5218x; 1.5218x over previous
"""Trainium2 Bass kernel for the NeuralODE adjoint problem.

Key structure (validated vs reference in numpy):
- The reference scan's carry passes the *observed* Y[s] to the next segment,
  so all B*S = 128 (sample, segment) tasks are independent. 8 cores x 16
  units (2 samples x 8 segments), batched along the matmul free dimension.
- Forward RK4 runs in "H-space": with G = W2@W1 and w1b2 = W1^T b2,
  u_{i+1} = UB_m + TVB_j + (c*dt) . (G^T H_i), where TVB_j (host tile)
  carries tau*vt + b1 + (c_i+m)*dt*w1b2. One matmul per stage; the UB+TVB
  part is preloaded into PSUM by the DVE and the matmul accumulates on top.
- Backward integrates the augmented adjoint ODE in y-space, stacking stage
  values (y, H, Ehat, a) into wide SBUF stacks; gradients come from 8 PE
  transposes + a few matmuls at the end with (stage,unit) as contraction.
  Ehat = (w_i*dt/6)(1-H^2)(W2 a) with per-unit scales folded via host
  broadcast tiles; DA = W1@Ehat gives unit-uniform stage constants.
- Outputs are row-packed and split across DMA queues (SBUF->DRAM costs
  ~160ns per partition-row descriptor).
"""
import numpy as np

N, Hd, B, S, NS = 64, 128, 16, 8, 4
NU = 16          # units per core
NC = 8           # cores
WID = 256        # stack width = 16 slices * 16 units
RKW = np.array([1.0, 2.0, 2.0, 1.0], dtype=np.float32)
RKC = [0.0, 0.5, 0.5, 1.0]
ACONST = [3.0, 1.5, 3.0]  # 6*c_{i+1}/w_i for i=0,1,2

# column offsets inside the two constant blobs
C128 = {"w2": 0, "w1r": 64, "b1": 128, "i128": 129, "nsb": 257, "tbc": 289,
        "dt2h": 293, "dth": 309, "dt6h": 325}
W128 = 341
C64 = {"w2r": 0, "y1t2": 128, "dtbc": 144, "saw": 240, "b2dt": 496}
W64 = 512

_CACHE = {}


def _f32(x):
    return np.asarray(x, dtype=np.float32)


def _build_program():
    import contextlib
    import os

    import concourse.bacc as bacc
    import concourse.bass as bass
    import concourse.mybir as mybir
    import concourse.tile as tile

    F32 = mybir.dt.float32
    F32R = mybir.dt.float32r
    Alu = mybir.AluOpType
    Act = mybir.ActivationFunctionType
    ts = bass.ts

    use_f32r = os.environ.get("KERNEL_F32R", "1") == "1"
    FM = F32R if use_f32r else F32

    nc = bacc.Bacc("TRN2", target_bir_lowering=False, debug=False, num_devices=NC)

    def mm(out, lhsT, rhs, **kw):
        nc.tensor.matmul(out, lhsT, rhs, **kw)

    d_y0t = nc.dram_tensor("y0t", [64, NU], FM, kind="ExternalInput").ap()
    d_w1vt = nc.dram_tensor("w1vt", [65, 128], FM, kind="ExternalInput").ap()
    d_tvb = nc.dram_tensor("tvb", [128, WID], FM, kind="ExternalInput").ap()
    d_g = nc.dram_tensor("g", [128, 128], FM, kind="ExternalInput").ap()
    d_b128 = nc.dram_tensor("b128", [128, W128], FM, kind="ExternalInput").ap()
    d_b64 = nc.dram_tensor("b64", [64, W64], FM, kind="ExternalInput").ap()
    d_tb = nc.dram_tensor("tb", [1, WID], FM, kind="ExternalInput").ap()

    d_yhat = nc.dram_tensor("yhat", [64, NU], F32, kind="ExternalOutput").ap()
    d_gw1 = nc.dram_tensor("gw1", [64, 128], F32, kind="ExternalOutput").ap()
    d_gw2t = nc.dram_tensor("gw2t", [64, 128], F32, kind="ExternalOutput").ap()
    d_gcols = nc.dram_tensor("gcols", [4, 128], F32, kind="ExternalOutput").ap()

    with tile.TileContext(nc) as tc, contextlib.ExitStack() as ctx:
        cp = ctx.enter_context(tc.tile_pool(name="cp", bufs=1))

        # order matters: earliest dispatches feed the first matmuls
        Y0T = cp.tile([64, NU], FM)
        nc.sync.dma_start(Y0T[:], d_y0t)
        W1VT = cp.tile([65, 128], FM)
        nc.sync.dma_start(W1VT[:], d_w1vt)
        TVB = cp.tile([128, WID], FM)
        nc.scalar.dma_start(TVB[:], d_tvb)
        G = cp.tile([128, 128], FM)
        nc.scalar.dma_start(G[:], d_g)
        B128 = cp.tile([128, W128], FM)
        nc.sync.dma_start(B128[:], d_b128)
        B64 = cp.tile([64, W64], FM)
        nc.scalar.dma_start(B64[:], d_b64)
        YSB = cp.tile([65, WID], FM)
        nc.sync.dma_start(YSB[64:65, :], d_tb)

        def c128(key, w, p=128):
            o = C128[key]
            return B128[0:p, o:o + w]

        W2 = c128("w2", 64)
        W1R = c128("w1r", 64)
        B1 = c128("b1", 1)
        I128 = c128("i128", 128)
        I64 = c128("i128", 64, p=64)
        NS1 = c128("nsb", 16)
        NS2 = B128[:, C128["nsb"] + 16:C128["nsb"] + 32]
        TBC = c128("tbc", 4)
        DT2H = c128("dt2h", 16)
        DTH = c128("dth", 16)
        DT6H = c128("dt6h", 16)

        def c64(key, off, w):
            o = C64[key] + off
            return B64[:, o:o + w]

        W2R = c64("w2r", 0, 128)
        Y1T2 = c64("y1t2", 0, 16)
        B2 = B64[:, C64["y1t2"] + 16:C64["y1t2"] + 17]
        DT6 = c64("dtbc", 32, 16)
        BDT2 = c64("dtbc", 48, 16)
        BDT = c64("dtbc", 64, 16)
        BDT6 = c64("dtbc", 80, 16)
        SAW = c64("saw", 0, 256)
        B2DT = c64("b2dt", 0, 16)

        HST = cp.tile([128, WID], FM)
        EST = cp.tile([128, WID], FM)
        AST = cp.tile([64, WID], FM)

        with tc.tile_pool(name="ps", bufs=1, space="PSUM") as ps, \
             tc.tile_pool(name="wk", bufs=3) as wk, \
             tc.tile_pool(name="kp", bufs=6) as kp, \
             tc.tile_pool(name="hp", bufs=6) as hp, \
             tc.tile_pool(name="sp", bufs=2) as sp:

            # ---------------- forward (H-space) ----------------
            UB0P = ps.tile([128, NU], F32, tag="gsum", bufs=1)
            mm(UB0P[:], W1VT[0:64, :], Y0T[:], start=True, stop=True)
            UBcur = sp.tile([128, NU], FM, tag="ub", bufs=2)
            nc.scalar.activation(UBcur[:], UB0P[:], Act.Copy)
            y0cur = Y0T

            CDTH = [DT2H, DT2H, DTH]
            for m in range(NS):
                # stage-0 psum: UB + TVB via DVE write; stages 1-3 preloaded
                ubank = []
                for i in range(4):
                    u_i = ps.tile([128, NU], F32, tag="u", bufs=3)
                    nc.vector.tensor_tensor(
                        u_i[:], UBcur[:], TVB[:, ts(4 * m + i, NU)], op=Alu.add)
                    ubank.append(u_i)
                RS = None
                Hts = []
                for i in range(4):
                    if i > 0:
                        Hs = wk.tile([128, NU], FM, tag="hs")
                        nc.vector.tensor_tensor(
                            Hs[:], Hts[i - 1][:], CDTH[i - 1], op=Alu.mult)
                        mm(ubank[i][:], G[:], Hs[:], start=False, stop=True,
                           skip_group_check=True)
                    H_i = hp.tile([128, NU], FM, tag="h")
                    nc.scalar.activation(H_i[:], ubank[i][:], Act.Tanh)
                    Hts.append(H_i)
                    if i == 0:
                        RS = H_i
                    elif i < 3:
                        RS2 = sp.tile([128, NU], FM, tag="rs", bufs=2)
                        nc.vector.scalar_tensor_tensor(
                            RS2[:], H_i[:], 2.0, RS[:], Alu.mult, Alu.add)
                        RS = RS2
                    else:
                        HSUM = sp.tile([128, NU], FM, tag="hsum", bufs=2)
                        nc.vector.tensor_tensor(HSUM[:], RS[:], H_i[:],
                                                op=Alu.add)
                # boundary: UB update + y0 update
                GSUM = ps.tile([128, NU], F32, tag="gsum", bufs=1)
                mm(GSUM[:], G[:], HSUM[:], start=True, stop=True)
                KSUM = ps.tile([64, NU], F32, tag="f", bufs=1)
                mm(KSUM[:], W2, HSUM[:], start=True, stop=True)
                if m < 3:
                    t1 = wk.tile([128, NU], FM, tag="ubt")
                    nc.vector.tensor_tensor(t1[:], GSUM[:], DT6H, op=Alu.mult)
                    UBnew = sp.tile([128, NU], FM, tag="ub", bufs=2)
                    nc.vector.tensor_tensor(UBnew[:], UBcur[:], t1[:],
                                            op=Alu.add)
                    UBcur = UBnew
                t2 = wk.tile([64, NU], FM, tag="y0t1")
                nc.vector.tensor_tensor(t2[:], KSUM[:], DT6, op=Alu.mult)
                t3 = wk.tile([64, NU], FM, tag="y0t2")
                nc.vector.tensor_tensor(t3[:], y0cur[:], t2[:], op=Alu.add)
                if m < 3:
                    y0new = sp.tile([64, NU], FM, tag="y0", bufs=2)
                    nc.vector.tensor_tensor(y0new[:], t3[:], B2DT, op=Alu.add)
                    y0cur = y0new
                else:
                    nc.vector.tensor_tensor(YSB[0:64, 0:NU], t3[:], B2DT,
                                            op=Alu.add)

            for q in range(4):
                eng = nc.sync if q % 2 == 0 else nc.scalar
                eng.dma_start(d_yhat[16 * q:16 * q + 16, :],
                              YSB[16 * q:16 * q + 16, 0:NU].bitcast(F32))
            # a0 = 2*yhat - 2*y1_obs
            nc.vector.scalar_tensor_tensor(
                AST[:, 0:NU], YSB[0:64, 0:NU], 2.0, Y1T2, Alu.mult,
                Alu.subtract)

            # ---------------- backward (y-space) ----------------
            for m in range(NS):
                base_y = YSB[0:64, ts(4 * m, NU)]
                base_a = AST[:, ts(4 * m, NU)]
                if m < 3:
                    DACC = ps.tile([64, NU], F32, tag="dacc")
                ks = []
                for i in range(4):
                    sl = 4 * m + i
                    s16 = ts(sl, NU)
                    U = ps.tile([128, NU], F32, tag="u", bufs=3)
                    mm(U[:], W1VT[:], YSB[:, s16], start=True, stop=True)
                    nc.scalar.activation(HST[:, s16], U[:], Act.Tanh, bias=B1)
                    Cc = ps.tile([128, NU], F32, tag="c")
                    mm(Cc[:], W2R, AST[:, s16], start=True, stop=True)
                    HH = wk.tile([128, NU], FM, tag="hh")
                    nc.vector.tensor_tensor(HH[:], HST[:, s16], HST[:, s16],
                                            op=Alu.mult)
                    nE = wk.tile([128, NU], FM, tag="ne")
                    nc.vector.scalar_tensor_tensor(
                        nE[:], HH[:], 1.0, Cc[:], Alu.subtract, Alu.mult)
                    nc.vector.tensor_tensor(
                        EST[:, s16], nE[:], NS1 if i in (0, 3) else NS2,
                        op=Alu.mult)
                    if i < 3 or m < 3:
                        Fp = ps.tile([64, NU], F32, tag="f")
                        mm(Fp[:], W2, HST[:, s16], start=True, stop=True)
                        k = kp.tile([64, NU], FM, tag="k")
                        nc.scalar.activation(k[:], Fp[:], Act.Identity,
                                             bias=B2)
                        ks.append(k)
                    if i < 3:
                        DA = ps.tile([64, NU], F32, tag="da")
                        mm(DA[:], W1R, EST[:, s16], start=True, stop=True)
                    if m < 3:
                        mm(DACC[:], W1R, EST[:, s16],
                           start=(i == 0), stop=(i == 3),
                           skip_group_check=True)
                    if i < 3:
                        tmp = wk.tile([64, NU], FM, tag="tmp")
                        nc.vector.tensor_tensor(
                            tmp[:], k[:], BDT2 if i < 2 else BDT, op=Alu.mult)
                        nc.vector.tensor_tensor(
                            YSB[0:64, ts(sl + 1, NU)], tmp[:], base_y,
                            op=Alu.add)
                        nc.vector.scalar_tensor_tensor(
                            AST[:, ts(sl + 1, NU)], DA[:], ACONST[i], base_a,
                            Alu.mult, Alu.add)
                if m < 3:
                    s1 = wk.tile([64, NU], FM, tag="s1")
                    nc.vector.tensor_tensor(s1[:], ks[0][:], ks[3][:],
                                            op=Alu.add)
                    s2 = wk.tile([64, NU], FM, tag="s2")
                    nc.vector.tensor_tensor(s2[:], ks[1][:], ks[2][:],
                                            op=Alu.add)
                    ss = wk.tile([64, NU], FM, tag="ss")
                    nc.vector.scalar_tensor_tensor(
                        ss[:], s2[:], 2.0, s1[:], Alu.mult, Alu.add)
                    t4 = wk.tile([64, NU], FM, tag="t4")
                    nc.vector.tensor_tensor(t4[:], ss[:], BDT6, op=Alu.mult)
                    nc.vector.tensor_tensor(
                        YSB[0:64, ts(4 * (m + 1), NU)], t4[:], base_y,
                        op=Alu.add)
                    nc.vector.tensor_tensor(
                        AST[:, ts(4 * (m + 1), NU)], base_a, DACC[:],
                        op=Alu.add)

        # ---------------- gradients ----------------
        with tc.tile_pool(name="pg", bufs=1, space="PSUM") as pg, \
             tc.tile_pool(name="um", bufs=1) as um:
            ASW = um.tile([64, WID], FM)
            nc.vector.tensor_tensor(ASW[:], AST[:], SAW, op=Alu.mult)
            GB1 = um.tile([128, 1], F32)
            nc.vector.tensor_reduce(GB1[:], EST[:], axis=mybir.AxisListType.X,
                                    op=Alu.add)
            GB2 = um.tile([64, 1], F32)
            nc.vector.tensor_reduce(GB2[:], ASW[:], axis=mybir.AxisListType.X,
                                    op=Alu.add)

            yum, eum, hum, aum = [], [], [], []
            for c in range(2):
                cols = ts(c, 128)
                TY = pg.tile([128, 64], FM, tag="tp", bufs=2)
                nc.tensor.transpose(TY[:], YSB[0:64, cols], I64)
                Y_ = um.tile([128, 64], FM, tag="yum", bufs=2)
                nc.scalar.activation(Y_[:], TY[:], Act.Copy)
                yum.append(Y_)
                TE = pg.tile([128, 128], FM, tag="tp2", bufs=2)
                nc.tensor.transpose(TE[:], EST[:, cols], I128)
                E_ = um.tile([128, 128], FM, tag="eum", bufs=2)
                nc.scalar.activation(E_[:], TE[:], Act.Copy)
                eum.append(E_)
                TH = pg.tile([128, 128], FM, tag="tp2", bufs=2)
                nc.tensor.transpose(TH[:], HST[:, cols], I128)
                H_ = um.tile([128, 128], FM, tag="hum", bufs=2)
                nc.scalar.activation(H_[:], TH[:], Act.Copy)
                hum.append(H_)
                TA = pg.tile([128, 64], FM, tag="tp", bufs=2)
                nc.tensor.transpose(TA[:], ASW[:, cols], I64)
                A_ = um.tile([128, 64], FM, tag="aum", bufs=2)
                nc.scalar.activation(A_[:], TA[:], Act.Copy)
                aum.append(A_)

            GW1P = pg.tile([64, 128], F32, tag="gw1")
            GW2TP = pg.tile([64, 128], F32, tag="gw2t")
            GVTP = pg.tile([128, 2], F32, tag="gvt")
            for c in range(2):
                mm(GW1P[:], yum[c][:], eum[c][:],
                   start=(c == 0), stop=(c == 1))
                mm(GW2TP[:], aum[c][:], hum[c][:],
                   start=(c == 0), stop=(c == 1))
                mm(GVTP[:], eum[c][:], TBC[:, 2 * c:2 * c + 2],
                   start=(c == 0), stop=(c == 1))

            # pack gvt/gb1/gb2 columns -> one (4,128) row tile
            GCOL = um.tile([128, 4], FM)
            nc.scalar.activation(GCOL[:, 0:1], GVTP[:, 0:1], Act.Copy)
            nc.scalar.activation(GCOL[:, 1:2], GB1[:], Act.Copy)
            nc.scalar.activation(GCOL[0:64, 2:3], GB2[:], Act.Copy)
            nc.vector.memset(GCOL[0:64, 3:4].bitcast(F32), 0.0)
            nc.vector.memset(GCOL[64:128, 2:4].bitcast(F32), 0.0)
            TGC = pg.tile([4, 128], FM, tag="tgc")
            nc.tensor.transpose(TGC[:], GCOL[:], I128)
            GROW = um.tile([4, 128], F32)
            nc.scalar.activation(GROW[:], TGC[:], Act.Copy)
            nc.sync.dma_start(d_gcols, GROW[:])

            GW1S = um.tile([64, 128], F32)
            nc.scalar.activation(GW1S[:], GW1P[:], Act.Copy)
            GW2TS = um.tile([64, 128], F32)
            nc.scalar.activation(GW2TS[:], GW2TP[:], Act.Copy)
            for q in range(4):
                r = slice(16 * q, 16 * q + 16)
                eng = nc.sync if q % 2 == 0 else nc.scalar
                eng.dma_start(d_gw1[r, :], GW1S[r, :])
                eng2 = nc.scalar if q % 2 == 0 else nc.sync
                eng2.dma_start(d_gw2t[r, :], GW2TS[r, :])

    nc.compile()
    return nc


def _host_prep(Ys, Ts, W1, b1, vt, W2, b2):
    """Build per-core input maps (fp32 exact, mirroring reference rounding)."""
    Ys, Ts = _f32(Ys), _f32(Ts)
    W1, b1, vt, W2, b2 = map(_f32, (W1, b1, vt, W2, b2))

    w1vt = np.vstack([W1, vt[None, :]])                        # (65,128)
    G = _f32(W2 @ W1)                                          # (128,128)
    w1b2 = _f32(W1.T @ b2)                                     # (128,)

    in_maps = []
    for core in range(NC):
        bs = [2 * core, 2 * core + 1]
        y0 = np.zeros((NU, N), np.float32)
        y1 = np.zeros((NU, N), np.float32)
        t0 = np.zeros(NU, np.float32)
        t1 = np.zeros(NU, np.float32)
        for ib, b in enumerate(bs):
            for s in range(S):
                u = ib * S + s
                sp = max(s - 1, 0)
                y0[u], t0[u] = Ys[b, sp], Ts[b, sp]
                y1[u], t1[u] = Ys[b, s], Ts[b, s]
        dt = _f32((t1 - t0) / NS)
        dtb = _f32(-dt)

        def stage_times(tstart, step):
            out = np.zeros((NS, 4, NU), np.float32)
            tcur = tstart.copy()
            half = _f32(step * _f32(0.5))
            for m in range(NS):
                out[m, 0] = tcur
                out[m, 1] = _f32(tcur + half)
                out[m, 2] = out[m, 1]
                out[m, 3] = _f32(tcur + step)
                tcur = _f32(tcur + step)
            return out

        tfm = stage_times(t0, dt)                              # (NS,4,NU)
        tbm = stage_times(t1, dtb)
        tb = tbm.reshape(1, WID)
        tbc = np.zeros((128, 4), np.float32)
        tbc[:, 0] = tb.reshape(2, 128)[0]
        tbc[:, 2] = tb.reshape(2, 128)[1]

        # TVB: tau*vt + b1 + (c_i + m)*dt*w1b2   per slice j=4m+i
        tvb = np.zeros((128, WID), np.float32)
        wdt = np.outer(w1b2, dt)                               # (128,NU)
        for m in range(NS):
            for i in range(4):
                j = 4 * m + i
                tvb[:, 16 * j:16 * j + 16] = _f32(
                    np.outer(vt, tfm[m, i]) + b1[:, None]
                    + _f32(RKC[i] + m) * wdt)

        dt6 = _f32(dt / 6)
        blocks = [_f32(dt * _f32(0.5)), dt, dt6,
                  _f32(dtb * _f32(0.5)), dtb, _f32(dtb / 6)]
        dtbc = np.concatenate(
            [np.broadcast_to(x, (N, NU)) for x in blocks], axis=1)  # (64,96)
        saw = np.concatenate(
            [np.broadcast_to(_f32(RKW[i] * dt6), (N, NU))
             for m in range(NS) for i in range(4)], axis=1)        # (64,256)
        ns1 = np.broadcast_to(_f32(-dt6), (Hd, NU))
        ns2 = np.broadcast_to(_f32(-dt6 * 2), (Hd, NU))

        b128 = np.zeros((128, W128), np.float32)
        b128[:, C128["w2"]:C128["w2"] + 64] = W2
        b128[:, C128["w1r"]:C128["w1r"] + 64] = W1.T
        b128[:, C128["b1"]] = b1
        b128[:, C128["i128"]:C128["i128"] + 128] = np.eye(128, dtype=np.float32)
        b128[:, C128["nsb"]:C128["nsb"] + 16] = ns1
        b128[:, C128["nsb"] + 16:C128["nsb"] + 32] = ns2
        b128[:, C128["tbc"]:C128["tbc"] + 4] = tbc
        b128[:, C128["dt2h"]:C128["dt2h"] + 16] = np.broadcast_to(
            _f32(dt * _f32(0.5)), (Hd, NU))
        b128[:, C128["dth"]:C128["dth"] + 16] = np.broadcast_to(dt, (Hd, NU))
        b128[:, C128["dt6h"]:C128["dt6h"] + 16] = np.broadcast_to(
            dt6, (Hd, NU))

        b64 = np.zeros((64, W64), np.float32)
        b64[:, C64["w2r"]:C64["w2r"] + 128] = W2.T
        b64[:, C64["y1t2"]:C64["y1t2"] + 16] = _f32(2.0 * y1.T)
        b64[:, C64["y1t2"] + 16] = b2
        b64[:, C64["dtbc"]:C64["dtbc"] + 96] = dtbc
        b64[:, C64["saw"]:C64["saw"] + 256] = saw
        b64[:, C64["b2dt"]:C64["b2dt"] + 16] = np.outer(b2, dt)

        in_maps.append({
            "b128": b128, "b64": b64, "w1vt": w1vt, "tvb": tvb, "g": G,
            "y0t": np.ascontiguousarray(y0.T), "tb": tb,
        })
    return in_maps


def _run(inputs, trace=False, tmpdir=None):
    from concourse.bass_utils import run_bass_kernel_spmd

    if "nc" not in _CACHE:
        _CACHE["nc"] = _build_program()
    nc = _CACHE["nc"]
    in_maps = _host_prep(**inputs)
    res = run_bass_kernel_spmd(nc, in_maps, list(range(NC)), trace=trace,
                               tmpdir=tmpdir)

    Ys_hat = np.zeros((B, S, N), np.float32)
    gW1 = np.zeros((N, Hd), np.float32)
    gW2 = np.zeros((Hd, N), np.float32)
    gvt = np.zeros(Hd, np.float32)
    gb1 = np.zeros(Hd, np.float32)
    gb2 = np.zeros(N, np.float32)
    for c in range(NC):
        r = res.results[c]
        Ys_hat[2 * c:2 * c + 2] = r["yhat"].T.reshape(2, S, N)
        gW1 += r["gw1"]
        gW2 += r["gw2t"].T
        gvt += r["gcols"][0]
        gb1 += r["gcols"][1]
        gb2 += r["gcols"][2][:64]
    return (Ys_hat, gW1, gb1, gvt, gW2, gb2), res


def kernel(Ys, Ts, W1, b1, vt, W2, b2):
    out, _ = _run(dict(Ys=Ys, Ts=Ts, W1=W1, b1=b1, vt=vt, W2=W2, b2=b2))
    return out
